# revision 12
# baseline (speedup 1.0000x reference)
"""Trainium2 Bass kernel for nn_KnowledgeCircuit (moe_routing).

  h   = einsum('bsd,ndr,bsn->bsr', x, feature_know, feature_know_w)
  out = einsum('bsr,bsn,nrd->bsd', h, restore_know_w, restore_know)

Shapes: B=4, S=2048, D=1024, N=64, R=128.

Sharding: data-parallel over the B*S = 8192 tokens -> 1024 tokens per
NeuronCore across 8 cores; the neuron pools (fk, rk) are replicated.
No collectives.

All matmuls run in bf16 (fp32 operands are cast on the Scalar engine
on-chip; fp32r measures 2 cycles/row on HW, bf16 1 cycle/row, and the
2e-2 rel-err budget is ~7x above bf16's error).

Host-side layout prep (pure reshape/transpose, no arithmetic):
  - x is passed transposed and d-interleaved: row j*128+p of xT holds
    x[:, 8p+j], so both stage-1 matmul operands use the d = 8p+j
    contraction mapping and every DMA line is 4KB contiguous.
  - fk is passed as the byte-identical view [N, 128, 8*128] (d = 8p+j).
  - w2 is passed transposed [N, T] for the per-pool row broadcast.
  - out is produced d-major [D, T]; the host transposes it back.

Per-core program:
  stage 1: for each quad of 4 pools: DMA fk pool-major, cast to bf16;
           psum[t128, 512] accumulates xT.T @ fk over 8 d-tiles;
           scalar_tensor_tensor applies the routing weight w1[:, n]
           and accumulates h[t, r] in fp32.
  stage 1.5: PE-transpose h -> hT, cast to bf16.
  stage 2: per pool n: DMA partition-broadcast of the bf16 w2^T row,
           g_n = hT * w2bc (DVE, bf16, kept in SBUF for both d-phases);
           PSUM accumulates rk-slices.T @ g_n over all 64 pools into
           8 banks [d128, t512] per d-half; drain DMAs psum directly
           to the d-major output.
"""

from contextlib import ExitStack

import numpy as np

import concourse.mybir as mybir
import concourse.tile as tile
from concourse import bacc
from concourse.bass_utils import run_bass_kernel_spmd
from concourse.masks import make_identity

F32 = mybir.dt.float32
BF16 = mybir.dt.bfloat16
MULT = mybir.AluOpType.mult
ADD = mybir.AluOpType.add

B, S, D, N, R = 4, 2048, 1024, 64, 128
N_CORES = 8
T = B * S // N_CORES  # tokens per core


def build_kernel(T=1024, D=1024, N=64, R=128, debug=False):
    """Build the per-core Bass program. T tokens per core."""
    assert T % 512 == 0 and D % 1024 == 0 and R == 128 and N % 4 == 0
    TT = T // 128          # token tiles
    DJ = D // 128          # d sub-tiles (stage-1 contraction)
    NQ = N // 4            # stage-1 quads (4 pools each, rhs 512 wide)
    T5 = T // 512          # 512-wide token tiles for stage 2
    DH = D // 2            # stage-2 d-half

    nc = bacc.Bacc(None, target_bir_lowering=False, debug=debug)

    # xT row j*128+p = x[:, 8p+j]
    xT_d = nc.dram_tensor("xT", [D, T], F32, kind="ExternalInput")
    w1_d = nc.dram_tensor("w1", [T, N], F32, kind="ExternalInput")
    w2T_d = nc.dram_tensor("w2T", [N, T], F32, kind="ExternalInput")
    # fk[n, p, j, r] = feature_know[n, 8p+j, r] (byte-identical view)
    fk_d = nc.dram_tensor("fk", [N, 128, DJ, 128], F32, kind="ExternalInput")
    rk_d = nc.dram_tensor("rk", [N, R, D], F32, kind="ExternalInput")
    out_d = nc.dram_tensor("out", [D, T], F32, kind="ExternalOutput")

    with tile.TileContext(nc) as tc, ExitStack() as ctx:
        sb_const = ctx.enter_context(tc.tile_pool(name="const", bufs=1))
        psum = ctx.enter_context(tc.tile_pool(name="ps", bufs=8, space="PSUM"))
        dram = ctx.enter_context(tc.tile_pool(name="dram", bufs=1, space="DRAM"))

        ident = sb_const.tile([128, 128], F32, tag="ident")
        make_identity(nc, ident[:])

        hT = sb_const.tile([128, T], BF16, tag="hT")

        # ---- stage 1: h[t, r] accumulation over all pools ----
        with ExitStack() as s1:
            sb_xst = s1.enter_context(tc.tile_pool(name="xst", bufs=4))
            sb_xT = s1.enter_context(tc.tile_pool(name="xTp", bufs=DJ))
            sb_w1 = s1.enter_context(tc.tile_pool(name="w1p", bufs=TT))
            sb_h = s1.enter_context(tc.tile_pool(name="hp", bufs=TT))
            sb_fst = s1.enter_context(tc.tile_pool(name="fst", bufs=2))
            sb_fk = s1.enter_context(tc.tile_pool(name="fkp", bufs=2))

            # quad-0 fk DMA first so its data is in flight during the xT loads
            fst0 = sb_fst.tile([128, 4, DJ, 128], F32, tag="fst")
            for i in range(4):
                nc.sync.dma_start(fst0[:, i, :, :], fk_d[i, :, :, :])
            fkq0 = sb_fk.tile([128, 4, DJ, 128], BF16, tag="fk")

            # interleave xT casts with per-j quad-0 fk casts on the Scalar
            # FIFO so the first matmul group is unblocked ASAP
            xT = []
            for j in range(DJ):
                xs = sb_xst.tile([128, T], F32, tag="xs")
                nc.sync.dma_start(xs[:], xT_d[j * 128 : (j + 1) * 128, :])
                xb = sb_xT.tile([128, T], BF16, tag="xT", name=f"xT{j}")
                nc.scalar.copy(xb[:], xs[:])
                nc.scalar.copy(fkq0[:, :, j, :], fst0[:, :, j, :])
                xT.append(xb)

            w1 = []
            for tt in range(TT):
                t1 = sb_w1.tile([128, N], F32, tag="w1")
                nc.sync.dma_start(t1[:], w1_d[tt * 128 : (tt + 1) * 128, :])
                w1.append(t1)

            h = [sb_h.tile([128, R], F32, tag="h", name=f"h{i}") for i in range(TT)]
            for tt in range(TT):
                nc.vector.memset(h[tt][:], 0.0)

            for q in range(NQ):
                if q == 0:
                    fkq = fkq0
                else:
                    # pool-major staging: fst[p, i, j, r]
                    fst = sb_fst.tile([128, 4, DJ, 128], F32, tag="fst")
                    for i in range(4):
                        nc.sync.dma_start(fst[:, i, :, :], fk_d[q * 4 + i, :, :, :])
                    fkq = sb_fk.tile([128, 4, DJ, 128], BF16, tag="fk")
                    nc.scalar.copy(fkq[:], fst[:])
                for ttg in range((TT + 3) // 4):
                    tts = range(ttg * 4, min(ttg * 4 + 4, TT))
                    hps = {
                        tt: psum.tile([128, 512], F32, tag="ps", name=f"hps{tt}")
                        for tt in tts
                    }
                    for j in range(DJ):
                        for tt in tts:
                            nc.tensor.matmul(
                                hps[tt][:],
                                xT[j][:, tt * 128 : (tt + 1) * 128],
                                fkq[:, :, j, :],
                                start=(j == 0),
                                stop=(j == DJ - 1),
                            )
                    for tt in tts:
                        for i in range(4):
                            n = q * 4 + i
                            nc.vector.scalar_tensor_tensor(
                                h[tt][:],
                                hps[tt][:, i * 128 : (i + 1) * 128],
                                w1[tt][:, n : n + 1],
                                h[tt][:],
                                MULT,
                                ADD,
                            )

            # w2^T -> bf16 -> DRAM (source for the per-pool partition_broadcast);
            # placed after the stage-1 casts so it doesn't head the Scalar FIFO
            w2Ts = sb_const.tile([N, T], F32, tag="w2Ts")
            nc.sync.dma_start(w2Ts[:], w2T_d[:, :])
            w2Tb = sb_const.tile([N, T], BF16, tag="w2Tb")
            nc.scalar.copy(w2Tb[:], w2Ts[:])
            w2T_dram = dram.tile([N, T], BF16, tag="w2Td")
            nc.sync.dma_start(w2T_dram[:], w2Tb[:])

            # ---- stage 1.5: hT (bf16) ----
            for tt in range(TT):
                tp = psum.tile([128, 128], F32, tag="ps")
                nc.tensor.transpose(tp[:], h[tt][:], ident[:])
                nc.vector.tensor_copy(hT[:, tt * 128 : (tt + 1) * 128], tp[:])

        # ---- stage 2: out accumulation over all pools, dk split in halves ----
        with ExitStack() as s2:
            sb_g = s2.enter_context(tc.tile_pool(name="gp", bufs=N))
            sb_bc = s2.enter_context(tc.tile_pool(name="bcp", bufs=4))
            sb_rst = s2.enter_context(tc.tile_pool(name="rst", bufs=6))
            sb_rk = s2.enter_context(tc.tile_pool(name="rkp", bufs=6))
            sb_ot = s2.enter_context(tc.tile_pool(name="otp", bufs=8))

            g = []
            for n in range(N):
                bc = sb_bc.tile([128, T], BF16, tag="bc")
                nc.sync.dma_start(
                    bc[:], w2T_dram[n : n + 1, :].partition_broadcast(128)
                )
                gn = sb_g.tile([128, T], BF16, tag="g", name=f"g{n}")
                nc.vector.tensor_mul(gn[:], hT[:], bc[:])
                g.append(gn)

            dkh = DH // 128  # d-tiles per half
            for ph in range(2):
                ops = [
                    psum.tile([128, 512], F32, tag="ps", name=f"ops{i}")
                    for i in range(dkh * T5)
                ]
                for n in range(N):
                    rst = sb_rst.tile([128, DH], F32, tag="rst")
                    nc.sync.dma_start(
                        rst[:], rk_d[n, :, ph * DH : (ph + 1) * DH]
                    )
                    rkh = sb_rk.tile([128, DH], BF16, tag="rk")
                    nc.scalar.copy(rkh[:], rst[:])
                    for dki in range(dkh):
                        for t5 in range(T5):
                            nc.tensor.matmul(
                                ops[dki * T5 + t5][:],
                                rkh[:, dki * 128 : (dki + 1) * 128],
                                g[n][:, t5 * 512 : (t5 + 1) * 512],
                                start=(n == 0),
                                stop=(n == N - 1),
                            )
                # drain: alternate Vector/Scalar so the tail copies run in
                # parallel on both engines
                for dki in range(dkh):
                    dk = ph * dkh + dki
                    for t5 in range(T5):
                        ot = sb_ot.tile([128, 512], F32, tag="ot")
                        if (dki * T5 + t5) % 2 == 0:
                            nc.vector.tensor_copy(ot[:], ops[dki * T5 + t5][:])
                        else:
                            nc.scalar.copy(ot[:], ops[dki * T5 + t5][:])
                        nc.sync.dma_start(
                            out_d[
                                dk * 128 : (dk + 1) * 128,
                                t5 * 512 : (t5 + 1) * 512,
                            ],
                            ot[:],
                        )

    nc.compile()
    return nc


_NC_CACHE = {}


def _get_nc():
    if "nc" not in _NC_CACHE:
        _NC_CACHE["nc"] = build_kernel(T=T, D=D, N=N, R=R, debug=False)
    return _NC_CACHE["nc"]


def _shard_inputs(x, feature_know_w, restore_know_w, feature_know, restore_know):
    xf = np.asarray(x, dtype=np.float32).reshape(B * S, D)
    w1f = np.asarray(feature_know_w, dtype=np.float32).reshape(B * S, N)
    w2f = np.asarray(restore_know_w, dtype=np.float32).reshape(B * S, N)
    # fk[n, p, j, r] = feature_know[n, 8p+j, r] -- byte-identical view
    fk = np.ascontiguousarray(np.asarray(feature_know, dtype=np.float32)).reshape(
        N, 128, D // 128, R
    )
    rk = np.ascontiguousarray(np.asarray(restore_know, dtype=np.float32))
    in_maps = []
    for c in range(N_CORES):
        sl = slice(c * T, (c + 1) * T)
        # xT row j*128+p = x[:, 8p+j]
        xTc = np.ascontiguousarray(
            xf[sl].T.reshape(128, D // 128, T).transpose(1, 0, 2).reshape(D, T)
        )
        in_maps.append(
            {
                "xT": xTc,
                "w1": np.ascontiguousarray(w1f[sl]),
                "w2T": np.ascontiguousarray(w2f[sl].T),
                "fk": fk,
                "rk": rk,
            }
        )
    return in_maps


def run(in_maps, **kwargs):
    nc = _get_nc()
    return run_bass_kernel_spmd(nc, in_maps, core_ids=list(range(N_CORES)), **kwargs)


def kernel(x, feature_know_w, restore_know_w, feature_know, restore_know, **_):
    in_maps = _shard_inputs(
        x, feature_know_w, restore_know_w, feature_know, restore_know
    )
    res = run(in_maps)
    out = np.concatenate([r["out"].T for r in res.results], axis=0)
    return out.reshape(B, S, D)
